# revision 29
# baseline (speedup 1.0000x reference)
"""Multi-head self-attention (CogView PB-relax variant) on 8 TRN2 NeuronCores.

Problem: B=2, S=2048, D=1024, H=16 heads, Dh=64.
  q/k/v = hidden @ W{q,k,v}.T + b          (per-head slices)
  scores = (q k^T + attn_bias) / 8 + (1-mask)*(-BIG)
  out    = softmax(scores) @ v             (PB-relax softmax == plain softmax)

Sharding: tensor-parallel over heads. Core c owns heads (2c, 2c+1) for both
batch rows: it reads full hidden, W-row slices [128c:128c+128], bias slice
[h=2c:2c+2], and writes output channels [128c:128(c+1)].

Device-side design (v8):
  - host pre-transposes / pre-casts raw inputs (pure layout work): hidden^T,
    W^T and bias^T arrive as bf16 DRAM tensors in the layouts the matmuls
    want.
  - phase 1: q^T/k^T/v^T projections (bf16 matmuls, contraction=D tiled by
    128); q^T/k^T kept [head-dim, token] resident in SBUF; q/k PSUM drains
    on DVE (tensor_scalar_add) so ACT stays free for phase-2 exp.
  - phase 2, per (q-block, batch, k-chunk): ONE 2-bank PSUM tile [128,2,512]
    holds both heads' transposed scores [k=128, q=512].  The attention bias
    is INJECTED through the PE (identity matmul, start=True) so no DVE
    bias-add is needed; the q·k matmuls accumulate on top (both heads packed
    in the PE via tile_position row groups).  A single ACT exp call
    (FD=1024, PSUM source) computes exp(x*0.125 + maskbias[k]) for both
    heads; AV accumulates ctx^T with lhsT = [v | 1] (65 cols) so row 64 is
    the masked softmax denominator.
  - epilogue: ctx^T [65,512] (data + denominator row) drains to SBUF on DVE,
    PE-transposes [65,128] -> [128,65] put tokens on partitions with the
    denominator in col 64; per-partition reciprocal + scale, store.
"""

import numpy as np
import ml_dtypes

import concourse.bass as bass
import concourse.mybir as mybir
import concourse.tile as tile
from concourse import bacc, bass_utils
from concourse.masks import make_identity

F32 = mybir.dt.float32
BF16 = mybir.dt.bfloat16
FP8 = mybir.dt.float8e4
I32 = mybir.dt.int32
Exp = mybir.ActivationFunctionType.Exp
DoubleRow = mybir.MatmulPerfMode.DoubleRow

B, S, D = 2, 2048, 1024
NCORES = 8
HPC = 2            # heads per core
OC = HPC * 64      # 128 output channels per core
QB = 512           # q block (free dim of score tiles)
NQB = S // QB      # 4
NKC = S // 128     # 16 k-chunks per batch row
NSB = (B * S) // 512   # 8 token blocks for projections
NDC = D // 128     # 8 contraction chunks

MASK_NEG = -30000.0
SCALE = 0.125


def _build_program():
    nc = bacc.Bacc(
        "TRN2", target_bir_lowering=False, debug=False, num_devices=NCORES
    )
    hidT = nc.dram_tensor("hid_t", [NSB, 128, NDC, 512], BF16,
                          kind="ExternalInput").ap()
    amask = nc.dram_tensor("attention_mask", [128, B, NKC], I32,
                           kind="ExternalInput").ap()
    biasT = nc.dram_tensor("bias_t", [HPC, NQB, 128, NKC, QB], BF16,
                           kind="ExternalInput").ap()
    wqt = nc.dram_tensor("wq_t", [128, NDC, OC], BF16, kind="ExternalInput").ap()
    wkt = nc.dram_tensor("wk_t", [128, NDC, OC], BF16, kind="ExternalInput").ap()
    wvt = nc.dram_tensor("wv_t", [128, NDC, OC], BF16, kind="ExternalInput").ap()
    bq = nc.dram_tensor("bq", [OC], F32, kind="ExternalInput").ap()
    bk = nc.dram_tensor("bk", [OC], F32, kind="ExternalInput").ap()
    bv = nc.dram_tensor("bv", [OC], F32, kind="ExternalInput").ap()
    out = nc.dram_tensor("out", [B, S, OC], F32, kind="ExternalOutput").ap()

    with tile.TileContext(nc) as tc:
        _attention(tc, out, hidT, amask, biasT,
                   [wqt, wkt, wvt], [bq, bk, bv])

    nc.compile()
    return nc


def _attention(tc, out, hidT, amask, biasT, ws, bs):
    nc = tc.nc

    with tc.tile_pool(name="singles", bufs=1) as singles:
        ident = singles.tile([128, 128], F32)    # for epilogue PE transposes
        make_identity(nc, ident)
        identb = singles.tile([128, 128], BF16)  # for PE bias-inject matmuls
        make_identity(nc, identb)

        # --- mask -> additive bias column layout [128, B, NKC] ------------
        mi = singles.tile([128, B, NKC], I32)
        nc.gpsimd.dma_start(out=mi, in_=amask)
        mf = singles.tile([128, B, NKC], F32)
        nc.vector.tensor_copy(out=mf, in_=mi)
        mb = singles.tile([128, B, NKC], F32)
        nc.vector.tensor_scalar(
            out=mb, in0=mf, scalar1=-MASK_NEG, scalar2=MASK_NEG,
            op0=mybir.AluOpType.mult, op1=mybir.AluOpType.add,
        )

        # --- projection bias vectors [128, 1] -----------------------------
        bvec = []
        for i, b_ap in enumerate(bs):
            t = singles.tile([128, 1], F32, tag=f"bvec{i}")
            nc.gpsimd.dma_start(out=t, in_=b_ap.rearrange("(p o) -> p o", o=1))
            bvec.append(t)

        # preload the exp table set so the first real exp doesn't pay ~2.7us
        warm = singles.tile([128, 1], F32)
        nc.vector.memset(warm, 0.0)
        nc.scalar.activation(out=warm, in_=warm, func=Exp)

        # --- persistent activations ---------------------------------------
        qt2 = [singles.tile([128, S], BF16, tag=f"qt2{bb}",
                           name=f"qt2{bb}") for bb in range(B)]
        kt2 = [singles.tile([128, S], BF16, tag=f"kt2{bb}",
                           name=f"kt2{bb}") for bb in range(B)]
        # va: [k-local, kc-pair, pair-half, h*66+d] fp8 for DoubleRow AV;
        # col 64/130 of each half holds the ones column (denominator trick)
        va = singles.tile([128, NKC, 2, 2 * 80], BF16, tag="va")
        nc.vector.memset(va[:, :, :, 64:65], 1.0)
        nc.vector.memset(va[:, :, :, 144:145], 1.0)

        # ============ phase 1: projections ================================
        # (btp opens before phase-1 pools so bias DMAs don't alias phase-1
        #  SBUF -- aliasing would serialize the qb0 bias load behind all of
        #  phase 1)
        with tc.tile_pool(name="b_t", bufs=4) as btp:
            with tc.tile_pool(name="h_t", bufs=3) as htp, \
                 tc.tile_pool(name="v_t", bufs=4) as vtp, \
                 tc.tile_pool(name="p_ps", bufs=4, space="PSUM") as pps:
                pend_vt2 = []
                wt3 = []
                for sb in range(NSB):
                    hts = htp.tile([128, NDC, 512], BF16, name="hts")
                    nc.sync.dma_start(out=hts, in_=hidT[sb])
                    if sb == 0:
                        # W^T tiles issued after the first hidden tile so
                        # the first projection matmul starts sooner
                        for i, w_ap in enumerate(ws):
                            t = singles.tile([128, NDC, 128], BF16,
                                             tag=f"wt{i}", name=f"wt{i}")
                            nc.sync.dma_start(out=t, in_=w_ap)
                            wt3.append(t)
                    for w in range(3):
                        pp = pps.tile([128, 512], F32)
                        for dc in range(NDC):
                            nc.tensor.matmul(
                                out=pp,
                                lhsT=wt3[w][:, dc, :],
                                rhs=hts[:, dc, :],
                                start=(dc == 0), stop=(dc == NDC - 1))
                        if w < 2:
                            dst = (qt2 if w == 0 else kt2)[sb // 4][
                                :, (sb % 4) * 512:(sb % 4 + 1) * 512]
                            nc.vector.tensor_scalar_add(
                                out=dst, in0=pp, scalar1=bvec[w])
                        else:
                            if sb % 2 == 0:
                                vt2 = vtp.tile([128, 2, 512], BF16,
                                               tag="vt2", name="vt2")
                                pend_vt2.append(vt2)
                            else:
                                vt2 = pend_vt2[-1]
                            nc.vector.tensor_scalar_add(
                                out=vt2[:, sb % 2, :], in0=pp, scalar1=bvec[2])
                            if sb % 2 == 1:
                                vts = vtp.tile([128, 8, 128], BF16,
                                               tag="vts", name="vts")
                                nc.scalar.dma_start(
                                    out=vts,
                                    in_=vt2.rearrange("p j q -> p (j q)"),
                                    transpose=True)
                                for j in range(8):
                                    kb = (sb - 1) * 4 + j
                                    for h in range(HPC):
                                        nc.gpsimd.tensor_copy(
                                            out=va[:, kb // 2, kb % 2,
                                                   h * 80:h * 80 + 64],
                                            in_=vts[:, j, h * 64:(h + 1) * 64])

            # ============ phase 2: attention ==============================
            with tc.tile_pool(name="pt", bufs=12) as ptp, \
                 tc.tile_pool(name="stage", bufs=2) as stp, \
                 tc.tile_pool(name="osb", bufs=3) as osp, \
                 tc.tile_pool(name="sc_ps", bufs=2, space="PSUM") as scp, \
                 tc.tile_pool(name="ctx_ps", bufs=4, space="PSUM") as cxp:

                def issue_ep_stage(ctx_):
                    # drain ctx^T (+denominator row) to SBUF on DVE
                    stage = [stp.tile([65, QB], F32, tag=f"st{h}",
                                      name=f"stage{h}") for h in range(HPC)]
                    for h in range(HPC):
                        nc.vector.tensor_copy(out=stage[h], in_=ctx_[h])
                    return stage

                def issue_epilogue(qb_, b_, stage):
                    # PE-transpose [65,128] -> [128,65] (tokens on
                    # partitions, denominator in col 64), recip+scale,
                    # store.  ONE psum allocation (one slot-steal): blocks
                    # 0-6 packed at 65-col pitch in bank 1, block 7 at col
                    # 512 (bank 2) so no matmul output crosses a bank.
                    osb = osp.tile([128, 4, 128], F32, tag="osb", name="osb")
                    for i2 in range(2):
                        tp = scp.tile([128, 2, HPC, 65], F32, tag="sc",
                                      name="ep_t")
                        rcp = stp.tile([128, 2, HPC], F32, tag="rcp",
                                       name="rcp")
                        for ii in range(2):
                            i = i2 * 2 + ii
                            for h in range(HPC):
                                nc.tensor.transpose(
                                    out=tp[:, ii, h, :],
                                    in_=stage[h][:, i * 128:(i + 1) * 128],
                                    identity=ident[0:65, 0:65])
                        for ii in range(2):
                            i = i2 * 2 + ii
                            for h in range(HPC):
                                nc.vector.reciprocal(
                                    out=rcp[:, ii, h:h + 1],
                                    in_=tp[:, ii, h, 64:65])
                                nc.vector.tensor_scalar_mul(
                                    out=osb[:, i, h * 64:(h + 1) * 64],
                                    in0=tp[:, ii, h, 0:64],
                                    scalar1=rcp[:, ii, h:h + 1])
                    nc.gpsimd.dma_start(
                        out=out[b_, qb_ * QB:(qb_ + 1) * QB, :]
                        .rearrange("(i p) k -> p i k", p=128),
                        in_=osb)

                def load_bt(qb_):
                    ts = []
                    for h in range(HPC):
                        t = btp.tile([128, NKC, QB], BF16, tag="bT",
                                     name=f"bt{h}")
                        nc.scalar.dma_start(out=t, in_=biasT[h, qb_])
                        ts.append(t)
                    return ts

                def issue_av(ctx_b, b_, pt_, kc_):
                    # AV for tile (b_, kc_): issued 2 tiles later (possibly
                    # inside the next batch/q-block), so it never exposes
                    # its exp dependency in the PE FIFO
                    gk = b_ * NKC + kc_
                    for h in range(HPC):
                        nc.tensor.matmul(
                            out=ctx_b[h],
                            lhsT=va[:, gk // 2, gk % 2,
                                    h * 80:h * 80 + 65],
                            rhs=pt_[:, h, :],
                            start=(kc_ == 0),
                            stop=(kc_ == NKC - 1))

                pend_av = []
                pend_ep = None
                bt_next = load_bt(0)
                for qb in range(NQB):
                    ctx = [[cxp.tile([65, QB], F32, tag="ctx",
                                     name=f"ctx{b}{h}")
                            for h in range(HPC)] for b in range(B)]
                    bt = bt_next
                    if qb + 1 < NQB:
                        bt_next = load_bt(qb + 1)
                    for b in range(B):
                        for kc in range(NKC):
                            sc = scp.tile([128, HPC, QB], F32, tag="sc",
                                          name="sc")
                            for h in range(HPC):
                                # bias -> PSUM through the PE (identity mm)
                                nc.tensor.matmul(
                                    out=sc[:, h, :], lhsT=identb,
                                    rhs=bt[h][:, kc, :],
                                    start=True, stop=False,
                                    skip_group_check=True)
                            for h in range(HPC):
                                nc.tensor.matmul(
                                    out=sc[:, h, :],
                                    lhsT=kt2[b][h * 64:(h + 1) * 64,
                                                kc * 128:(kc + 1) * 128],
                                    rhs=qt2[b][h * 64:(h + 1) * 64,
                                               qb * QB:(qb + 1) * QB],
                                    start=False, stop=True,
                                    tile_position=(h * 64, 0),
                                    skip_group_check=True)
                            pt = ptp.tile([128, HPC, QB], BF16,
                                          tag="pt", name="pt")
                            # one exp call for both heads from PSUM
                            nc.scalar.activation(
                                out=pt.rearrange("p h q -> p (h q)"),
                                in_=sc.rearrange("p h q -> p (h q)"),
                                func=Exp,
                                bias=mb[:, b, kc:kc + 1], scale=SCALE)
                            pend_av.append((ctx[b], b, pt, kc))
                            if len(pend_av) > 2:   # AV trails 2 tiles
                                issue_av(*pend_av.pop(0))
                            if kc == 2 and pend_ep is not None:
                                stage_ = issue_ep_stage(pend_ep[2])
                                pend_ep = (pend_ep[0], pend_ep[1], stage_)
                            if kc == 5 and pend_ep is not None:
                                issue_epilogue(*pend_ep)
                                pend_ep = None
                        pend_ep = (qb, b, ctx[b])
                for pa in pend_av:
                    issue_av(*pa)
                issue_epilogue(pend_ep[0], pend_ep[1],
                               issue_ep_stage(pend_ep[2]))


_CACHE = {}


def _get_program():
    if "nc" not in _CACHE:
        _CACHE["nc"] = _build_program()
    return _CACHE["nc"]


def _wprep(w):
    # [oc, D] -> [p, c, oc]: per-partition-contiguous for fast DMA
    bf = ml_dtypes.bfloat16
    return np.ascontiguousarray(
        w.T.reshape(D // 128, 128, OC).transpose(1, 0, 2)).astype(bf)


def _shard_inputs(inputs):
    """Host-side layout prep: transposes and bf16 casts only (no compute)."""
    bf = ml_dtypes.bfloat16
    hs = np.asarray(inputs["hidden_state"], dtype=np.float32)
    # [sb, p, c, s]: per-partition-contiguous 8KB runs for fast DMA
    hid_t = np.ascontiguousarray(
        hs.reshape(B * S, D).T.reshape(D // 128, 128, NSB, 512)
        .transpose(2, 1, 0, 3)).astype(bf)
    am = np.ascontiguousarray(
        np.asarray(inputs["attention_mask"], dtype=np.int32)
        .reshape(B, NKC, 128).transpose(2, 0, 1))
    ab = np.asarray(inputs["attention_bias"], dtype=np.float32)
    wts = {k: np.asarray(inputs[k], dtype=np.float32) for k in ("Wq", "Wk", "Wv")}
    vb = {k: np.ascontiguousarray(np.asarray(inputs[k], dtype=np.float32))
          for k in ("bq", "bk", "bv")}
    in_maps = []
    for c in range(NCORES):
        r0, r1 = c * OC, (c + 1) * OC
        # [h, qb, p, c, s]: bias[h, q=qb*512+s, k=c*128+p]
        bias_t = np.ascontiguousarray(
            ab[0, HPC * c:HPC * (c + 1)]
            .reshape(HPC, NQB, QB, NKC, 128)
            .transpose(0, 1, 4, 3, 2)).astype(bf)
        in_maps.append({
            "hid_t": hid_t,
            "attention_mask": am,
            "bias_t": bias_t,                                   # [h, k, q]
            "wq_t": _wprep(wts["Wq"][r0:r1]),
            "wk_t": _wprep(wts["Wk"][r0:r1]),
            "wv_t": _wprep(wts["Wv"][r0:r1]),
            "bq": vb["bq"][r0:r1],
            "bk": vb["bk"][r0:r1],
            "bv": vb["bv"][r0:r1],
        })
    return in_maps


def kernel(**inputs):
    nc = _get_program()
    in_maps = _shard_inputs(inputs)
    res = bass_utils.run_bass_kernel_spmd(
        nc, in_maps, core_ids=list(range(NCORES)))
    parts = [np.asarray(res.results[c]["out"]) for c in range(NCORES)]
    return np.concatenate(parts, axis=-1)


def run_profiled(inputs, trace=True):
    """test.py helper: returns (output, BassKernelResults)."""
    nc = _get_program()
    in_maps = _shard_inputs(inputs)
    res = bass_utils.run_bass_kernel_spmd(
        nc, in_maps, core_ids=list(range(NCORES)), trace=trace)
    parts = [np.asarray(res.results[c]["out"]) for c in range(NCORES)]
    return np.concatenate(parts, axis=-1), res


if __name__ == "__main__":
    # quick compile check
    _build_program()
    print("compile OK")
